# revision 24
# baseline (speedup 1.0000x reference)
"""Causal single-head attention (B=64, T=512, D=768, H=96) on 8 TRN2 NeuronCores.

Data-parallel: core c computes x[8c:8c+8] with replicated weights; no
collectives.

Per-batch dataflow (v2: software-pipelined across batches):
  x --PE-transpose (f32r, 24x 128x128, stride-6 column AP)--> xT[6p+d, t]
  qkv_nat[t-chunk, 0:288] = xT_chunk.T @ [Wq|Wk|Wv]   (f32r, N=288: 1 cyc/row)
  PSUM->SBUF copies emit bf16: qkv_nat [128, 289] with a ones column at 288
  qT/kT[h, t] via bf16 PE transposes (1.0 cyc/row)
  scoresT_j[tk, tq>=128j] = kT_j.T @ qT   (bf16: 1 cyc/row at any N)
  eT = ACT Exp(scale*scoresT) -> bf16; GPSIMD masks the diagonal block
  outT[0:97, tq] += v1_j.T @ eT_j   (v1 = qkv_nat[:, 192:289]; the ones
    column accumulates the softmax denominator in row 96)
  outT --PE-transpose (f32r)--> psum[tq, 0:97]; out = psum[:,:96]*recip(:,96)

PE executes in emission order, so the emission interleaves three pipeline
stages per slot s: prep(s) = transposes+projections (depends only on DMA
and its own copies), attn(s-1) = qkT/scores/out (depends on ACT/DVE copy
round-trips, which prep work hides), finish(s-2) = transpose-back +
normalize + store. Weights are DMA'd fully contiguously (rows 6p..6p+5 on
partition p, 2304B descriptors) on the SP queue behind x[0]; the stride-6
x-transpose AP makes contraction chunk d = rows {6p+d} match that layout.
"""

import numpy as np

import concourse.bass as bass
import concourse.mybir as mybir
import concourse.tile as tile
from concourse.masks import make_identity, make_upper_triangular

B, T, D, H = 64, 512, 768, 96
N_CORES = 8
BP = B // N_CORES  # batches per core
P = 128
DC = D // P  # 6 contraction chunks
TC = T // P  # 4 sequence chunks
W3 = 3 * H  # 288 packed projection columns
SCALE = 1.0 / float(np.sqrt(H))
F32 = mybir.dt.float32
F32R = mybir.dt.float32r
BF16 = mybir.dt.bfloat16

XSPLIT = 2  # x DMAs per batch (split along the sequence chunks)


def _r(ap):
    return ap.bitcast(F32R)


def _split_excess_waits(nc: bass.Bass, limit: int = 1) -> None:
    """This walrus build rejects instructions with more than one sync-wait
    command ("Too many sync wait commands" in setupSyncWait). Move excess
    waits onto preceding single-wait NoOps on the same engine — the engine
    processes instructions in order, so blocking semantics are preserved."""
    k = 0
    for f in nc.m.functions:
        for blk in f.blocks:
            out = []
            for inst in blk.instructions:
                si = inst.sync_info
                if si is not None and len(si.on_wait) > limit:
                    waits = sorted(
                        si.on_wait,
                        key=lambda w: ((w.ant_name or "").startswith("DMA"), ),
                    )
                    for w in waits[:-limit]:
                        nop = mybir.InstNoOp(name=f"WSPLIT-{k}", engine=inst.engine)
                        k += 1
                        nop.sync_info = mybir.SyncInfo(on_wait=[w], on_update=[])
                        out.append(nop)
                    inst.sync_info = mybir.SyncInfo(
                        on_wait=waits[-limit:], on_update=list(si.on_update)
                    )
                out.append(inst)
            blk.instructions = out


def build_bass(repeat: int = 1) -> bass.Bass:
    nc = bass.Bass(name="attn_dp")
    x = nc.dram_tensor("x", (BP, T, D), F32, kind="ExternalInput")
    wq = nc.dram_tensor("Wq", (D, H), F32, kind="ExternalInput")
    wk = nc.dram_tensor("Wk", (D, H), F32, kind="ExternalInput")
    wv = nc.dram_tensor("Wv", (D, H), F32, kind="ExternalInput")
    out = nc.dram_tensor("out", (BP, T, H), F32, kind="ExternalOutput")

    NB = BP * repeat

    with tile.TileContext(nc) as tc:
        with (
            tc.tile_pool(name="consts", bufs=1) as consts,
            tc.tile_pool(name="xin", bufs=3) as xin,
            tc.tile_pool(name="xtp", bufs=2) as xtp,
            tc.tile_pool(name="qkvp", bufs=8) as qkvp,
            tc.tile_pool(name="qkTp", bufs=2) as qkTp,
            tc.tile_pool(name="expp", bufs=2) as expp,
            tc.tile_pool(name="otp", bufs=2) as otp,
            tc.tile_pool(name="outp", bufs=8) as outp,
            tc.tile_pool(name="ps_xt", bufs=2, space="PSUM") as ps_xt,
            tc.tile_pool(name="ps_mid", bufs=2, space="PSUM") as ps_mid,
            tc.tile_pool(name="ps_qkt", bufs=1, space="PSUM") as ps_qkt,
            tc.tile_pool(name="ps_sc", bufs=2, space="PSUM") as ps_sc,
            tc.tile_pool(name="ps_o", bufs=1, space="PSUM") as ps_o,
        ):
            # ---- constants ----
            ident = consts.tile([P, P], F32)
            make_identity(nc, ident)
            ident_r = consts.tile([P, P], F32, tag="ident_r")
            nc.vector.tensor_copy(_r(ident_r), ident)
            ident_b = consts.tile([P, P], BF16, tag="ident_b")
            nc.vector.tensor_copy(ident_b, ident)
            # keep-mask for the diagonal block of scoresT[tk, tq]: 1 iff tk<=tq
            tri = consts.tile([P, P], F32)
            make_upper_triangular(nc, tri, val=1.0, diag=True)
            tri_b = consts.tile([P, P], BF16, tag="tri_b")
            nc.vector.tensor_copy(tri_b, tri)
            ones_b = consts.tile([P, 1], BF16, tag="ones_b")
            nc.gpsimd.memset(ones_b, 1.0)

            # ---- per-batch state ----
            x_tiles = {}
            xt_tiles = {}
            qkv_tiles = {}
            qkT_tiles = {}
            eT_tiles = {}
            qkT_ps = {}
            ops_tiles = {}
            ot_tiles = {}

            def load_x(b, nsplit=XSPLIT):
                x_sb = xin.tile([P, TC, D], F32)
                xr = x[b % BP].rearrange("(i p) d -> p i d", p=P)
                step = TC // nsplit
                for s in range(nsplit):
                    nc.sync.dma_start(
                        out=_r(x_sb[:, s * step : (s + 1) * step, :]),
                        in_=_r(xr[:, s * step : (s + 1) * step, :]),
                    )
                x_tiles[b] = x_sb

            load_x(0, nsplit=4)

            # weights: fully-contiguous load (partition p <- rows 6p..6p+5)
            # on the SP queue so they can't jump ahead of x[0]; one repack
            # copy each into interleaved [P, DC, 288]
            w_int = consts.tile([P, DC, W3], F32, tag="w_int")
            for widx, w in enumerate((wq, wk, wv)):
                w_cont = consts.tile([P, DC, H], F32, tag=f"wc{widx}")
                nc.sync.dma_start(
                    out=w_cont, in_=w.rearrange("(p o) h -> p o h", p=P)
                )
                nc.vector.tensor_copy(
                    _r(w_int[:, :, widx * H : (widx + 1) * H]), w_cont
                )

            # ---- pipeline stage emitters ----
            def emit_T(b, d):
                # transpose x d-chunk (stride-6 columns) -> xT[:, d, :]
                x_sb = x_tiles[b]
                if d == 0:
                    xt_sb = xtp.tile([P, DC, T], F32, tag="xt_sb")
                    xt_tiles[b] = xt_sb
                xt_ps = ps_xt.tile([P, T], F32, tag="xt")
                for i in range(TC):
                    src = x_sb[:, i, :].rearrange("p (t c) -> p t c", c=DC)
                    nc.tensor.transpose(
                        _r(xt_ps[:, i * P : (i + 1) * P]),
                        _r(src[:, :, d]),
                        _r(ident_r),
                    )
                if d < 2:
                    nc.scalar.copy(out=_r(xt_tiles[b][:, d, :]), in_=xt_ps)
                else:
                    nc.vector.tensor_copy(_r(xt_tiles[b][:, d, :]), xt_ps)
                if d == DC - 1:
                    x_tiles.pop(b)

            def emit_proj(b, c):
                if c == 0:
                    qkv_tiles[b] = []
                pp = ps_mid.tile([P, W3], F32, tag="proj")
                for d in range(DC):
                    nc.tensor.matmul(
                        pp,
                        lhsT=_r(xt_tiles[b][:, d, c * P : (c + 1) * P]),
                        rhs=_r(w_int[:, d, :]),
                        start=(d == 0),
                        stop=(d == DC - 1),
                    )
                qn = qkvp.tile([P, W3 + 1], BF16, tag=f"qkv{c}")
                nc.gpsimd.tensor_copy(out=qn[:, W3 : W3 + 1], in_=ones_b)
                if c % 2 == 0:
                    nc.vector.tensor_copy(qn[:, :W3], pp)
                else:
                    nc.scalar.copy(out=qn[:, :W3], in_=pp)
                qkv_tiles[b].append(qn)
                if c == TC - 1:
                    xt_tiles.pop(b)

            def emit_qkT(b, qi):
                # both q and k transposes land in one [H, 2, T] psum tile;
                # qi==1 issues the single SBUF copy (one DVE round trip)
                if qi == 0:
                    tp = ps_qkt.tile([H, 2, T], BF16, tag="qkT")
                    qkT_ps[b] = tp
                tp = qkT_ps[b]
                for c in range(TC):
                    nc.tensor.transpose(
                        tp[:, qi, c * P : (c + 1) * P],
                        qkv_tiles[b][c][:, qi * H : (qi + 1) * H],
                        ident_b,
                    )
                if qi == 1:
                    sb = qkTp.tile([H, 2, T], BF16, tag="qkT")
                    nc.vector.tensor_copy(sb, tp)
                    qkT_tiles[b] = (sb[:, 0, :], sb[:, 1, :])
                    qkT_ps.pop(b)

            def emit_sc(b, j):
                # scoresT chunk j + exp + diagonal causal mask
                if j == 0:
                    eT_tiles[b] = [None] * TC
                qT_sb, kT_sb = qkT_tiles[b]
                nj = T - j * P
                sc_ps = ps_sc.tile([P, T], F32, tag="sc")
                nc.tensor.matmul(
                    sc_ps[:, :nj],
                    lhsT=kT_sb[:, j * P : (j + 1) * P],
                    rhs=qT_sb[:, j * P :],
                    start=True,
                    stop=True,
                )
                et = expp.tile([P, nj], BF16, tag=f"exp{j}")
                nc.scalar.activation(
                    out=et,
                    in_=sc_ps[:, :nj],
                    func=mybir.ActivationFunctionType.Exp,
                    scale=SCALE,
                )
                nc.vector.tensor_tensor(
                    out=et[:, :P], in0=et[:, :P], in1=tri_b,
                    op=mybir.AluOpType.mult,
                )
                eT_tiles[b][j] = et

            def emit_out(b, j):
                # outT[0:97, tq] accumulation for tk chunk j; diag/off-diag
                # split so only the diag part waits on the causal mask
                if j == 0:
                    o_ps_new = ps_o.tile([H + 1, T], F32, tag="o")
                    ops_tiles[b] = o_ps_new
                o_ps = ops_tiles[b]
                v1 = qkv_tiles[b][j][:, 2 * H : W3 + 1]
                et = eT_tiles[b][j]
                # only the FIRST matmul of the group may carry start=True:
                # start clears has_written for the whole bank, so a second
                # start=True would make later accumulations overwrite
                if j < TC - 1:
                    nc.tensor.matmul(
                        o_ps[:, (j + 1) * P :],
                        lhsT=v1,
                        rhs=et[:, P:],
                        start=(j == 0),
                        stop=False,
                    )
                nc.tensor.matmul(
                    o_ps[:, j * P : (j + 1) * P],
                    lhsT=v1,
                    rhs=et[:, :P],
                    start=False,
                    stop=(j == TC - 1),
                )
                if j == TC - 1:
                    qkv_tiles.pop(b)
                    eT_tiles.pop(b)

            def emit_ot(b):
                ot_sb = otp.tile([H + 1, T], F32)
                o_ps = ops_tiles.pop(b)
                nc.scalar.copy(out=ot_sb[:, : T // 2], in_=o_ps[:, : T // 2])
                nc.vector.tensor_copy(ot_sb[:, T // 2 :], o_ps[:, T // 2 :])
                ot_tiles[b] = ot_sb

            def emit_fin(b, i, o_all):
                # transpose-back + normalize chunk i; DMA in pairs
                tr_ps = ps_sc.tile([P, H + 1], F32, tag="sc")
                nc.tensor.transpose(
                    tr_ps,
                    ot_tiles[b][:, i * P : (i + 1) * P],
                    ident[: H + 1, : H + 1],
                )
                rec = outp.tile([P, 1], F32, tag="rec")
                nc.vector.reciprocal(rec, tr_ps[:, H : H + 1])
                nc.vector.tensor_scalar_mul(o_all[:, i, :], tr_ps[:, :H], rec)
                if i % 2 == 1:
                    nc.sync.dma_start(
                        out=out[b % BP].rearrange("(i p) h -> p i h", p=P)[
                            :, i - 1 : i + 1, :
                        ],
                        in_=o_all[:, i - 1 : i + 1, :],
                    )
                if i == TC - 1:
                    ot_tiles.pop(b)

            # ---- software-pipelined emission ----
            # slot 0: prep(0) only (DMA-bound startup)
            if NB > 1:
                load_x(1)
            for d in range(DC):
                emit_T(0, d)
            for c in range(TC):
                emit_proj(0, c)

            for s in range(1, NB + 1):
                a = s - 1  # attention batch
                p = s if s < NB else None  # prep batch
                f = s - 2  # finish batch
                o_all = None
                if f >= 0:
                    o_all = outp.tile([P, TC, H], F32, tag="o_all")
                if p is not None and p + 1 < NB:
                    load_x(p + 1)
                # interleave: prep fills attn's copy-latency waits.
                # all T(p, 0..5) precede proj(p, 0) (projections contract
                # over every d chunk)
                if p is not None:
                    emit_T(p, 0)
                    emit_T(p, 1)
                emit_qkT(a, 0)
                if p is not None:
                    emit_T(p, 2)
                emit_qkT(a, 1)
                if p is not None:
                    emit_T(p, 3)
                emit_sc(a, 0)
                if p is not None:
                    emit_T(p, 4)
                    emit_T(p, 5)
                emit_out(a, 0)
                emit_sc(a, 1)
                if f >= 0:
                    emit_fin(f, 0, o_all)
                if p is not None:
                    emit_proj(p, 0)
                emit_out(a, 1)
                emit_sc(a, 2)
                if f >= 0:
                    emit_fin(f, 1, o_all)
                if p is not None:
                    emit_proj(p, 1)
                emit_out(a, 2)
                emit_sc(a, 3)
                if f >= 0:
                    emit_fin(f, 2, o_all)
                if p is not None:
                    emit_proj(p, 2)
                if f >= 0:
                    emit_fin(f, 3, o_all)
                if p is not None:
                    emit_proj(p, 3)
                emit_out(a, 3)
                emit_ot(a)

            # drain: finish the last batch
            for fb in (NB - 1,):
                o_all = outp.tile([P, TC, H], F32, tag="o_all")
                for i in range(TC):
                    emit_fin(fb, i, o_all)

    _split_excess_waits(nc)
    return nc


def kernel(x: np.ndarray, Wq: np.ndarray, Wk: np.ndarray, Wv: np.ndarray) -> np.ndarray:
    from concourse.bass_utils import run_bass_kernel_spmd

    x = np.ascontiguousarray(np.asarray(x, dtype=np.float32))
    Wq = np.ascontiguousarray(np.asarray(Wq, dtype=np.float32))
    Wk = np.ascontiguousarray(np.asarray(Wk, dtype=np.float32))
    Wv = np.ascontiguousarray(np.asarray(Wv, dtype=np.float32))

    in_maps = [
        {"x": x[c * BP : (c + 1) * BP], "Wq": Wq, "Wk": Wk, "Wv": Wv}
        for c in range(N_CORES)
    ]
    last_exc = None
    for attempt in range(3):
        try:
            nc = build_bass()
            res = run_bass_kernel_spmd(nc, in_maps, core_ids=list(range(N_CORES)))
            return np.concatenate([r["out"] for r in res.results], axis=0)
        except Exception as e:  # transient NRT/axon device errors
            last_exc = e
            import time as _time

            _time.sleep(2.0 * (attempt + 1))
    raise last_exc
